# revision 32
# baseline (speedup 1.0000x reference)
# Trainium2 Bass kernel for CausalSelfAttention (B=2, T=2048, C=1024, H=16, D=64)
# with periodic mask: causal AND (key_col % 4 != 3).
#
# Sharding (8 NeuronCores): core c = (b, g) with b = c//4 (batch), g = c%4
# (head group of 4 heads). Each core computes QKV for its 4 heads, attention,
# and a partial output projection y_heads @ Wp[rows]. Host sums the 4 partials
# per batch and adds bp (tensor-parallel reduce).
#
# Key device-side choices:
#  - Token permutation: each 512-token window is reordered on the host as
#    [384 kept | 128 dropped] (kept = t%4 != 3).  K and V projections run
#    only on the kept segment (25% less PE work, no gather matmuls), and the
#    kept x kept causal mask is EXACTLY lower-triangular in compacted
#    coordinates.  Q runs on both segments (two moving ranges into one PSUM
#    accumulation, same cycle count as dense).  The host un-permutes the
#    output rows at combine time.
#  - All DRAM inputs are pre-arranged on the host into the exact SBUF layout
#    (partition-major), so every bulk DMA is a contiguous per-partition
#    stream. The first weight/x pieces are split into k-chunk grains across
#    both HWDGE rings so the first QKV matmul can start as early as possible.
#  - Scores are produced transposed (S^T[tk_kept, tq]) so softmax-normalized
#    probabilities feed the AV matmul directly as the moving operand.
#  - Softmax row sums come from a 64-wide all-ones block in the V tiles; the
#    reciprocal runs on DVE (nc.vector.reciprocal), keeping the Act engine
#    free for the score exponentials (its true workload).
#  - Emission interleave: QKV chains of window j+1 and output-projection
#    pieces of window j-1 are woven between attention tiles of window j so
#    the PE keeps busy while the Act engine works through the exps.
#  - Output projection stores DMA straight from a bf16 stage.

import ml_dtypes
import numpy as np

B, T, C, H, D = 2, 2048, 1024, 16, 64
HG = 4          # heads per core
CG = HG * D     # = 256 columns of C per core
TK = (T // 4) * 3   # 1536 kept key positions
NTK = TK // 128     # 12 kept-key chunks of 128
KW = 384            # kept tokens per 512-token window
DW = 128            # dropped tokens per window
SCALE = 1.0 / 8.0   # 1/sqrt(D)

_CACHE = {}


def _split_multi_waits(nc, mybir):
    # The pinned walrus here encodes at most 1 sync-wait per instruction
    # (2 for EventSemaphore). Hoist excess waits onto standalone NoOps that
    # precede the instruction on the same engine.
    f = nc.m.functions[0]
    n = 0
    for b in f.blocks:
        insts = list(b.instructions)
        out = []
        changed = False
        for inst in insts:
            si = inst.sync_info
            if si is not None:
                waits = list(si.on_wait)
                cap = 2 if isinstance(inst, mybir.InstEventSemaphore) else 1
                if len(waits) > cap:
                    for w in waits[cap:]:
                        out.append(mybir.InstNoOp(
                            name=f"{inst.name}-ws{n}", engine=inst.engine,
                            ins=[], outs=[],
                            sync_info=mybir.SyncInfo(on_wait=[w], on_update=[])))
                        n += 1
                    inst.sync_info = mybir.SyncInfo(
                        on_wait=waits[:cap], on_update=list(si.on_update))
                    changed = True
            out.append(inst)
        if changed:
            b.instructions = out
    return n


def _build_bass(split=True):
    import contextlib
    import concourse.bass as bass
    import concourse.tile as tile
    import concourse.mybir as mybir

    f32 = mybir.dt.float32
    bf16 = mybir.dt.bfloat16

    nc = bass.Bass("TRN2", debug=False, num_devices=8)

    # host-prearranged, partition-major
    xk_d = nc.dram_tensor("xk", [4, 128, 8, KW], bf16, kind="ExternalInput").ap()
    xd_d = nc.dram_tensor("xd", [4, 128, 8, DW], bf16, kind="ExternalInput").ap()
    wq_d = nc.dram_tensor("wq", [128, 2, 8, 128], bf16, kind="ExternalInput").ap()
    wk_d = nc.dram_tensor("wk", [128, 2, 8, 128], bf16, kind="ExternalInput").ap()
    wv_d = nc.dram_tensor("wv", [128, 8, CG], bf16, kind="ExternalInput").ap()
    wp_d = nc.dram_tensor("wp", [128, 2, C], bf16, kind="ExternalInput").ap()
    bq_d = nc.dram_tensor("bq2", [128, 2], f32, kind="ExternalInput").ap()
    bk_d = nc.dram_tensor("bk2", [128, 2], f32, kind="ExternalInput").ap()
    bvb_d = nc.dram_tensor("bvb", [128, HG, D], f32, kind="ExternalInput").ap()
    cmk_d = nc.dram_tensor("cmk", [128, 2, 128], bf16, kind="ExternalInput").ap()
    cmd_d = nc.dram_tensor("cmd", [128, 3, 2, 128], bf16, kind="ExternalInput").ap()
    out_d = nc.dram_tensor("out", [T, C], bf16, kind="ExternalOutput").ap()

    Exp = mybir.ActivationFunctionType.Exp
    MULT = mybir.AluOpType.mult

    with tile.TileContext(nc) as tc, \
         tc.tile_pool(name="persist", bufs=1) as persist, \
         tc.tile_pool(name="work", bufs=1) as work, \
         tc.tile_pool(name="ps_a", space="PSUM", bufs=2) as ps_a, \
         tc.tile_pool(name="ps_s", space="PSUM", bufs=2) as ps_s, \
         tc.tile_pool(name="ps_y", space="PSUM", bufs=2) as ps_y:
        # ---------- persistent SBUF ----------
        qt = [persist.tile([128, T], bf16, name=f"qt{m}", tag=f"qt{m}") for m in range(2)]
        kt = [persist.tile([128, TK], bf16, name=f"kt{m}", tag=f"kt{m}") for m in range(2)]
        vsb = persist.tile([128, NTK, HG, 2 * D], bf16, name="vsb", tag="vsb")
        yt = [persist.tile([128, T], bf16, name=f"yt{m}", tag=f"yt{m}") for m in range(2)]
        cmk = persist.tile([128, 2, 128], bf16, name="cmk", tag="cmk")
        cmd = persist.tile([128, 3, 2, 128], bf16, name="cmd", tag="cmd")
        bqs = persist.tile([128, 2], f32, name="bqs", tag="bqs")
        bks = persist.tile([128, 2], f32, name="bks", tag="bks")
        bvb = persist.tile([128, HG, D], f32, name="bvb", tag="bvb")
        wp_t = persist.tile([128, 2, C], bf16, name="wp_t", tag="wp_t")
        wq_t = persist.tile([128, 2, 8, 128], bf16, name="wq_t", tag="wq_t")
        wk_t = persist.tile([128, 2, 8, 128], bf16, name="wk_t", tag="wk_t")
        wv_t = persist.tile([128, 8, CG], bf16, name="wv_t", tag="wv_t")

        # ones block for the AV row sums: generated on-chip
        nc.vector.memset(vsb[:, :, :, D:2 * D], 1.0)

        # pre-load the Act spline tables while the engines wait for data
        # (the first real exp otherwise eats the ~1.3us ACT_TABLE_LOAD)
        actw = work.tile([128, 2], f32, tag="actw", bufs=1)
        nc.vector.memset(actw[:, 0:1], 0.0)
        nc.scalar.activation(actw[:, 1:2], actw[:, 0:1], Exp)

        # SWDGE ring: small constants + V weights + masks (the software DGE
        # has its own descriptor path and surprisingly high burst bandwidth,
        # so it serves as a third lane for window-0-critical bytes)
        nc.gpsimd.dma_start(bqs[:], bq_d[:])
        nc.gpsimd.dma_start(bks[:], bk_d[:])
        nc.gpsimd.dma_start(bvb[:], bvb_d[:])
        nc.gpsimd.dma_start(wv_t[:], wv_d[:])
        nc.gpsimd.dma_start(wk_t[:, 0], wk_d[:, 0])
        nc.gpsimd.dma_start(cmk[:], cmk_d[:])
        nc.gpsimd.dma_start(cmd[:], cmd_d[:])

        # HBM is heavily contended (8 cores start identical DMA streams at
        # once): stream the head-critical bytes across both HWDGE rings in
        # first-use order, with few triggers (each DMA_DIRECT2D costs the
        # issuing engine ~0.6us of NX time).  Per-window x tiles (no ring
        # rotation) keep the trigger engines from blocking on pool reuse.
        xk = []
        xd = []
        for j in range(4):
            xk.append(work.tile([128, 8, KW], bf16, name=f"xk{j}",
                                tag=f"xkw{j}", bufs=1))
            xd.append(work.tile([128, 8, DW], bf16, name=f"xd{j}",
                                tag=f"xdw{j}", bufs=1))
        for k in range(0, 8, 2):
            nc.sync.dma_start(xk[0][:, k:k + 2, :], xk_d[0, :, k:k + 2, :])
        nc.scalar.dma_start(wq_t[:, 0, 0:4], wq_d[:, 0, 0:4])
        nc.scalar.dma_start(xd[0][:, 0:4, :], xd_d[0, :, 0:4, :])
        nc.scalar.dma_start(wq_t[:, 0, 4:8], wq_d[:, 0, 4:8])
        nc.scalar.dma_start(xd[0][:, 4:8, :], xd_d[0, :, 4:8, :])
        # window-0 critical stream ends here; the rest follows in need order
        # mid-kernel loads ride the sync ring only: the scalar-engine NX is
        # the exp pipeline, and every DMA_DIRECT2D costs it ~0.6us
        nc.scalar.dma_start(wk_t[:, 1], wk_d[:, 1])
        nc.sync.dma_start(wq_t[:, 1], wq_d[:, 1])
        nc.sync.dma_start(xk[1][:, 0:4, :], xk_d[1, :, 0:4, :])
        nc.sync.dma_start(xk[1][:, 4:8, :], xk_d[1, :, 4:8, :])
        nc.sync.dma_start(xd[1][:], xd_d[1])
        nc.sync.dma_start(wp_t[:], wp_d[:])
        nc.sync.dma_start(xk[2][:, 0:4, :], xk_d[2, :, 0:4, :])
        nc.sync.dma_start(xk[2][:, 4:8, :], xk_d[2, :, 4:8, :])
        nc.sync.dma_start(xd[2][:], xd_d[2])
        nc.sync.dma_start(xk[3][:, 0:4, :], xk_d[3, :, 0:4, :])
        nc.sync.dma_start(xk[3][:, 4:8, :], xk_d[3, :, 4:8, :])
        nc.sync.dma_start(xd[3][:], xd_d[3])

        # ---------- deferred-emission helpers (PE filler work) ----------
        def emit_q(j, m, pool=None):
            # kept segment [0:KW] and dropped segment [KW:512], one PSUM
            # accumulation group (start on very first, stop on very last)
            pq = (pool or ps_a).tile([128, 512], f32, tag="acc")
            for k in range(8):
                nc.tensor.matmul(pq[:, 0:KW], wq_t[:, m, k, :],
                                 xk[j][:, k, :], start=(k == 0), stop=False)
                nc.tensor.matmul(pq[:, KW:512], wq_t[:, m, k, :],
                                 xd[j][:, k, :], start=False, stop=(k == 7))
            nc.vector.tensor_scalar_add(qt[m][:, 512 * j:512 * (j + 1)],
                                        pq[:], bqs[:, m:m + 1])

        def emit_k(j, m, pool=None):
            # kept tokens only
            pk = (pool or ps_a).tile([128, 512], f32, tag="acc")
            for k in range(8):
                nc.tensor.matmul(pk[:, 0:KW], wk_t[:, m, k, :],
                                 xk[j][:, k, :], start=(k == 0), stop=(k == 7))
            nc.vector.tensor_scalar_add(kt[m][:, KW * j:KW * (j + 1)],
                                        pk[:, 0:KW], bks[:, m:m + 1])

        def emit_v(j, mm, pool=None):
            # kept-token chunk mm as stationary -> vsb chunk 3j+mm directly
            pv = (pool or ps_a).tile([128, 512], f32, tag="acc")
            for k in range(8):
                nc.tensor.matmul(pv[:, 0:CG],
                                 xk[j][:, k, 128 * mm:128 * (mm + 1)],
                                 wv_t[:, k, :], start=(k == 0), stop=(k == 7))
            nc.vector.scalar_tensor_tensor(
                out=vsb[:, 3 * j + mm, :, 0:D],
                in0=pv[:, 0:CG].rearrange("p (h d) -> p h d", d=D),
                scalar=1.0, in1=bvb[:],
                op0=mybir.AluOpType.bypass, op1=mybir.AluOpType.add)

        def emit_qkv_items(j, pools=(None,)):
            fns = [
                lambda p: emit_q(j, 0, p),
                lambda p: emit_k(j, 0, p),
                lambda p: emit_v(j, 0, p),
                lambda p: emit_v(j, 1, p),
                lambda p: emit_q(j, 1, p),
                lambda p: emit_k(j, 1, p),
                lambda p: emit_v(j, 2, p),
            ]
            return [lambda fn=fn, p=pools[ix % len(pools)]: fn(p)
                    for ix, fn in enumerate(fns)]

        def emit_proj(m):
            # output projection for token chunk m; bf16 staged, bf16 store
            stage = work.tile([128, C], bf16, tag="stage", bufs=2)
            for n in range(2):
                po = ps_a.tile([128, 512], f32, tag="acc")
                for k2 in range(2):
                    nc.tensor.matmul(
                        po[:], yt[k2][:, 128 * m:128 * (m + 1)],
                        wp_t[:, k2, 512 * n:512 * (n + 1)],
                        start=(k2 == 0), stop=(k2 == 1))
                nc.vector.tensor_copy(stage[:, 512 * n:512 * (n + 1)], po[:])
            if m % 2:
                ring = nc.gpsimd if m < 12 else nc.scalar
                ring.dma_start(out_d[128 * m:128 * (m + 1), :], stage[:])
            else:
                nc.sync.dma_start(out_d[128 * m:128 * (m + 1), :], stage[:])

        # window-3 projection is split by k2 so only half the matmuls sit
        # behind the final softmax-norm: the k2=0 halves (ready once hp0's
        # norm lands mid-window) run as hp1-pass filler into fp32 stages
        pstage = {}

        def emit_proj_half(m, n):
            po = ps_a.tile([128, 512], f32, tag="acc")
            nc.tensor.matmul(po[:], yt[0][:, 128 * m:128 * (m + 1)],
                             wp_t[:, 0, 512 * n:512 * (n + 1)],
                             start=True, stop=True)
            st = work.tile([128, 512], f32, tag=f"pst{m}_{n}", bufs=1)
            nc.vector.tensor_copy(st[:], po[:])
            pstage[(m, n)] = st

        def emit_proj_fin(m):
            stage = work.tile([128, C], bf16, tag="stage", bufs=2)
            for n in range(2):
                pool = ps_y if n else ps_a
                po = pool.tile([128, 512], f32, tag="pyo" if n else "acc")
                nc.tensor.matmul(po[:], yt[1][:, 128 * m:128 * (m + 1)],
                                 wp_t[:, 1, 512 * n:512 * (n + 1)],
                                 start=True, stop=True)
                nc.vector.scalar_tensor_tensor(
                    out=stage[:, 512 * n:512 * (n + 1)], in0=po[:],
                    scalar=1.0, in1=pstage[(m, n)][:],
                    op0=mybir.AluOpType.bypass, op1=mybir.AluOpType.add)
                ring = nc.sync if (m + n) % 2 == 0 else nc.scalar
                ring.dma_start(out_d[128 * m:128 * (m + 1),
                                     512 * n:512 * (n + 1)],
                               stage[:, 512 * n:512 * (n + 1)])

        # ---------- main schedule ----------
        pending = []
        post_norm = []  # deferred softmax-norm emission from previous window

        def drain_one():
            if pending:
                pending.pop(0)()

        def emit_norm2(hp, scr, jw):
            # deferred norm from an SBUF copy of the AV accumulators: both
            # heads' rowsums in one Ln and one Exp (halves the Act bubble,
            # and keeps the norm out of the producing window's exp stream)
            rec = work.tile([64, 2, 512], f32, tag="rec2", bufs=3)
            lns = work.tile([64, 2, 512], f32, tag="lns2", bufs=3)
            nc.scalar.activation(lns[:], scr[64:128, :, :],
                                 mybir.ActivationFunctionType.Ln)
            nc.scalar.activation(rec[:], lns[:], Exp, bias=0.0, scale=-1.0)
            for q in range(2):
                nc.vector.tensor_tensor(
                    yt[hp][64 * q:64 * q + 64, jw],
                    scr[0:64, q, :], rec[:, q, :], op=MULT)

        def evac_norm(hp, pys, jw, tag):
            # copy the AV accumulators out of PSUM (frees the banks) and
            # return a closure that emits the Act/DVE norm ops later
            scr = work.tile([128, 2, 512], f32, tag=tag, bufs=2)
            for q in range(2):
                nc.vector.tensor_copy(scr[:, q, :], pys[hp][q][:])
            return lambda s=scr, h=hp, j2=jw: emit_norm2(h, s, j2)

        # window 0: emit only what attention(hp0, tile0) needs inline; defer
        # the rest into the attention slots (2 filler drains per tile in
        # window 0)
        w0 = emit_qkv_items(0)
        for it in w0[:3]:
            it()
        pending.extend(w0[3:])

        for j in range(4):
            if j == 0:
                pending.extend(emit_qkv_items(1))
            elif j < 3:
                # qkv filler first (needed by next window's attention), then
                # any deferred output-projection pieces
                pending[0:0] = emit_qkv_items(j + 1)

            jwin = slice(512 * j, 512 * (j + 1))
            ntile = 3 * (j + 1)
            nb0 = ntile - 3  # first boundary tile index
            pys = {}

            def emit_avs(hp, i, pt2, avo):
                for q in range(2):
                    nc.tensor.matmul(
                        pys[hp][q][:, avo:512], vsb[:, i, 2 * hp + q, :],
                        pt2[:, q, avo:512],
                        start=(i == 0), stop=(i == ntile - 1))

            # software-pipelined over a flat (hp, i) tile list: QK(t) and the
            # filler overlap exp(t) on Act; AV trails by one tile
            prev = None
            my_norms = []
            tiles = [(hp, i) for hp in range(2) for i in range(ntile)]
            # emit the previous window's deferred norms a couple of exp
            # tiles into this window so their Act ops don't delay the first
            # exps (window 3 drains yt-readers from tile 0, so emit at 0)
            for tix, (hp, i) in enumerate(tiles):
                if j == 3:
                    if tix == 0:
                        while post_norm:
                            post_norm.pop(0)()
                elif tix in (2, 4) and post_norm:
                    post_norm.pop(0)()
                if i == 0:
                    pys[hp] = [ps_y.tile([128, 512], f32,
                                         name=f"py{j}_{hp}_{q}", tag="pyo")
                               for q in range(2)]
                ps2 = ps_s.tile([128, 2, 512], f32, tag="ps2")
                pt2 = work.tile([128, 2, 512], bf16, tag="pt2", bufs=4)
                u = i - nb0
                # boundary tiles: kept-query prefix [0:128u) is fully masked
                off = 128 * u if u >= 1 else 0
                # first tiles of a window: raise priority so their QK/exp
                # beat the previous window's stragglers and hide the norm
                hoist = (tc.high_priority(offset=150)
                         if (j > 0 and tix < 2) else contextlib.nullcontext())
                with hoist:
                    for q in range(2):  # q: row group (head 2*hp + q)
                        nc.tensor.matmul(
                            ps2[:, q, off:512],
                            kt[hp][64 * q:64 * q + 64, 128 * i:128 * (i + 1)],
                            qt[hp][64 * q:64 * q + 64,
                                   512 * j + off:512 * (j + 1)],
                            start=True, stop=True)
                    nc.scalar.activation(pt2[:, :, off:512],
                                         ps2[:, :, off:512],
                                         Exp, bias=0.0, scale=SCALE)
                if u == 2:  # u=2 AV stays full width: zero skipped cols
                    nc.gpsimd.memset(pt2[:, :, 0:off], 0.0)
                if u >= 0:  # boundary tile: causal mask (both heads at once)
                    # kept partial block [128u:128u+128) is the SAME
                    # triangular mask in compacted coords for every u; the
                    # dropped block [KW:512) is per-u from the host
                    nc.vector.tensor_tensor(
                        pt2[:, :, 128 * u:128 * u + 128],
                        pt2[:, :, 128 * u:128 * u + 128],
                        cmk[:], op=MULT)
                    nc.vector.tensor_tensor(
                        pt2[:, :, KW:512], pt2[:, :, KW:512],
                        cmd[:, u], op=MULT)
                drain_one()
                if j == 0:
                    drain_one()
                if prev is not None:
                    emit_avs(*prev)
                    if prev[1] == ntile - 1:
                        my_norms.append(evac_norm(0, pys, jwin, "scr0"))
                prev = (hp, i, pt2, 128 if u == 1 else 0)
            emit_avs(*prev)
            my_norms.append(evac_norm(1, pys, jwin, "scr1"))
            if j < 3:
                post_norm.extend(my_norms)
            else:
                # final window: norms run behind the last exp; the k2=0
                # projection halves fill the PE while the norms resolve
                for fn in my_norms:
                    fn()
                for m in range(12, 16):
                    for n in range(2):
                        emit_proj_half(m, n)
            while pending:
                drain_one()
            # ---- output projection for the finished query window ----
            if j < 3:
                pending.extend(
                    [lambda m=m: emit_proj(m) for m in range(4 * j, 4 * j + 4)])
            else:
                for m in range(12, 16):
                    emit_proj_fin(m)

    if split:
        _split_multi_waits(nc, mybir)
    return nc


def _get_nc():
    if "nc" not in _CACHE:
        _CACHE["nc"] = _build_bass()
    return _CACHE["nc"]


def _tokperm():
    # per-window permutation: [384 kept | 128 dropped] -> original index
    p = np.arange(512)
    kept = (p // 3) * 4 + p % 3          # for p < 384
    drop = (p - 384) * 4 + 3             # for p >= 384
    perm = np.where(p < KW, kept, drop)
    full = (np.arange(4)[:, None] * 512 + perm[None, :]).reshape(-1)
    return full  # [T] device row r holds token full[r]


def _host_maps(inputs):
    x = np.asarray(inputs["x"], np.float32)
    Wq = np.asarray(inputs["Wq"], np.float32)
    Wk = np.asarray(inputs["Wk"], np.float32)
    Wv = np.asarray(inputs["Wv"], np.float32)
    Wp = np.asarray(inputs["Wp"], np.float32)
    bq = np.asarray(inputs["bq"], np.float32)
    bk = np.asarray(inputs["bk"], np.float32)
    bv = np.asarray(inputs["bv"], np.float32)

    # causal masks in permuted-window coordinates: the kept x kept partial
    # block is the same lower-triangular mask (in compacted coords) for
    # every boundary chunk u; the dropped-query block differs per u.
    kp = np.arange(128)
    dq = np.arange(DW) * 4 + 3  # dropped-query original (window-local) index
    tri = (kp[None, :] >= kp[:, None]).astype(np.float32)
    cmk = np.zeros((128, 2, 128), np.float32)
    cmk[:, 0] = tri
    cmk[:, 1] = tri
    cmd = np.zeros((128, 3, 2, 128), np.float32)
    for u in range(3):
        ko = ((128 * u + kp) // 3) * 4 + (128 * u + kp) % 3  # key orig index
        dm = (dq[None, :] >= ko[:, None]).astype(np.float32)
        cmd[:, u, 0] = dm
        cmd[:, u, 1] = dm

    # permuted, compacted x: xk = kept tokens, xd = dropped tokens, both
    # partition-major [window, 128 cin-in-chunk, 8 cin-chunk, tokens]
    keep = np.arange(T) % 4 != 3
    xks, xds = [], []
    for b in range(B):
        xw = x[b].reshape(4, 512, C)
        xkb = np.stack([xw[j][keep[:512]] for j in range(4)])   # [4,384,C]
        xdb = np.stack([xw[j][~keep[:512]] for j in range(4)])  # [4,128,C]
        xks.append(np.ascontiguousarray(
            xkb.transpose(0, 2, 1).reshape(4, 8, 128, KW).transpose(0, 2, 1, 3)
        ).astype(ml_dtypes.bfloat16))
        xds.append(np.ascontiguousarray(
            xdb.transpose(0, 2, 1).reshape(4, 8, 128, DW).transpose(0, 2, 1, 3)
        ).astype(ml_dtypes.bfloat16))
    maps = []
    for c in range(8):
        b, g = c // 4, c % 4
        sl = slice(CG * g, CG * (g + 1))
        maps.append({
            "xk": xks[b],
            "xd": xds[b],
            "wq": np.ascontiguousarray(
                Wq[:, sl].reshape(8, 128, 2, 128).transpose(1, 2, 0, 3)
            ).astype(ml_dtypes.bfloat16),
            "wk": np.ascontiguousarray(
                Wk[:, sl].reshape(8, 128, 2, 128).transpose(1, 2, 0, 3)
            ).astype(ml_dtypes.bfloat16),
            "wv": np.ascontiguousarray(
                Wv[:, sl].reshape(8, 128, CG).transpose(1, 0, 2)
            ).astype(ml_dtypes.bfloat16),
            "wp": np.ascontiguousarray(
                Wp[sl, :].reshape(2, 128, C).transpose(1, 0, 2)
            ).astype(ml_dtypes.bfloat16),
            "bq2": np.ascontiguousarray(bq[sl].reshape(2, 128).T),
            "bk2": np.ascontiguousarray(bk[sl].reshape(2, 128).T),
            "bvb": np.ascontiguousarray(
                np.broadcast_to(bv[sl].reshape(HG, D), (128, HG, D))),
            "cmk": cmk.astype(ml_dtypes.bfloat16),
            "cmd": cmd.astype(ml_dtypes.bfloat16),
        })
    return maps


def _combine(results, inputs):
    bp = np.asarray(inputs["bp"], np.float32)
    perm = _tokperm()
    out = np.zeros((B, T, C), np.float32)
    for c in range(8):
        out[c // 4, perm] += np.asarray(results[c]["out"]).astype(np.float32)
    out += bp[None, None, :]
    return out


def _run(inputs, profile_dir=None, trace_cores=None):
    nc = _get_nc()
    maps = _host_maps(inputs)
    from concourse.bass_utils import run_bass_kernel_spmd
    if profile_dir is not None:
        import types, sys
        from trn_agent_boot.trn_boot import _ntff_profile_via_ctypes
        hook = _ntff_profile_via_ctypes("/opt/axon/libaxon_pjrt.so")
        with hook(profile_dir, trace_cores or [0]):
            res = run_bass_kernel_spmd(nc, maps, core_ids=list(range(8)))
    else:
        res = run_bass_kernel_spmd(nc, maps, core_ids=list(range(8)))
    return _combine(res.results, inputs)


def kernel(**inputs):
    return _run(inputs)


# revision 33
# speedup vs baseline: 1.0053x; 1.0053x over previous
# Trainium2 Bass kernel for CausalSelfAttention (B=2, T=2048, C=1024, H=16, D=64)
# with periodic mask: causal AND (key_col % 4 != 3).
#
# Sharding (8 NeuronCores): core c = (b, g) with b = c//4 (batch), g = c%4
# (head group of 4 heads). Each core computes QKV for its 4 heads, attention,
# and a partial output projection y_heads @ Wp[rows]. Host sums the 4 partials
# per batch and adds bp (tensor-parallel reduce).
#
# Key device-side choices:
#  - Token permutation: each 512-token window is reordered on the host as
#    [384 kept | 128 dropped] (kept = t%4 != 3).  K and V projections run
#    only on the kept segment (25% less PE work, no gather matmuls), and the
#    kept x kept causal mask is EXACTLY lower-triangular in compacted
#    coordinates.  Q runs on both segments (two moving ranges into one PSUM
#    accumulation, same cycle count as dense).  The host un-permutes the
#    output rows at combine time.
#  - All DRAM inputs are pre-arranged on the host into the exact SBUF layout
#    (partition-major), so every bulk DMA is a contiguous per-partition
#    stream. The first weight/x pieces are split into k-chunk grains across
#    both HWDGE rings so the first QKV matmul can start as early as possible.
#  - Scores are produced transposed (S^T[tk_kept, tq]) so softmax-normalized
#    probabilities feed the AV matmul directly as the moving operand.
#  - Softmax row sums come from a 64-wide all-ones block in the V tiles; the
#    reciprocal runs on DVE (nc.vector.reciprocal), keeping the Act engine
#    free for the score exponentials (its true workload).
#  - Emission interleave: QKV chains of window j+1 and output-projection
#    pieces of window j-1 are woven between attention tiles of window j so
#    the PE keeps busy while the Act engine works through the exps.
#  - Output projection stores DMA straight from a bf16 stage.

import ml_dtypes
import numpy as np

B, T, C, H, D = 2, 2048, 1024, 16, 64
HG = 4          # heads per core
CG = HG * D     # = 256 columns of C per core
TK = (T // 4) * 3   # 1536 kept key positions
NTK = TK // 128     # 12 kept-key chunks of 128
KW = 384            # kept tokens per 512-token window
DW = 128            # dropped tokens per window
SCALE = 1.0 / 8.0   # 1/sqrt(D)

_CACHE = {}


def _split_multi_waits(nc, mybir):
    # The pinned walrus here encodes at most 1 sync-wait per instruction
    # (2 for EventSemaphore). Hoist excess waits onto standalone NoOps that
    # precede the instruction on the same engine.
    f = nc.m.functions[0]
    n = 0
    for b in f.blocks:
        insts = list(b.instructions)
        out = []
        changed = False
        for inst in insts:
            si = inst.sync_info
            if si is not None:
                waits = list(si.on_wait)
                cap = 2 if isinstance(inst, mybir.InstEventSemaphore) else 1
                if len(waits) > cap:
                    for w in waits[cap:]:
                        out.append(mybir.InstNoOp(
                            name=f"{inst.name}-ws{n}", engine=inst.engine,
                            ins=[], outs=[],
                            sync_info=mybir.SyncInfo(on_wait=[w], on_update=[])))
                        n += 1
                    inst.sync_info = mybir.SyncInfo(
                        on_wait=waits[:cap], on_update=list(si.on_update))
                    changed = True
            out.append(inst)
        if changed:
            b.instructions = out
    return n


def _build_bass(split=True):
    import contextlib
    import concourse.bass as bass
    import concourse.tile as tile
    import concourse.mybir as mybir

    f32 = mybir.dt.float32
    bf16 = mybir.dt.bfloat16

    nc = bass.Bass("TRN2", debug=False, num_devices=8)

    # host-prearranged, partition-major
    xk_d = nc.dram_tensor("xk", [4, 128, 8, KW], bf16, kind="ExternalInput").ap()
    xd_d = nc.dram_tensor("xd", [4, 128, 8, DW], bf16, kind="ExternalInput").ap()
    wq_d = nc.dram_tensor("wq", [128, 2, 8, 128], bf16, kind="ExternalInput").ap()
    wk_d = nc.dram_tensor("wk", [128, 2, 8, 128], bf16, kind="ExternalInput").ap()
    wv_d = nc.dram_tensor("wv", [128, 8, CG], bf16, kind="ExternalInput").ap()
    wp_d = nc.dram_tensor("wp", [128, 2, C], bf16, kind="ExternalInput").ap()
    bq_d = nc.dram_tensor("bq2", [128, 2], f32, kind="ExternalInput").ap()
    bk_d = nc.dram_tensor("bk2", [128, 2], f32, kind="ExternalInput").ap()
    bvb_d = nc.dram_tensor("bvb", [128, HG, D], f32, kind="ExternalInput").ap()
    cmk_d = nc.dram_tensor("cmk", [128, 2, 128], bf16, kind="ExternalInput").ap()
    cmd_d = nc.dram_tensor("cmd", [128, 3, 2, 128], bf16, kind="ExternalInput").ap()
    out_d = nc.dram_tensor("out", [T, C], bf16, kind="ExternalOutput").ap()

    Exp = mybir.ActivationFunctionType.Exp
    MULT = mybir.AluOpType.mult

    with tile.TileContext(nc) as tc, \
         tc.tile_pool(name="persist", bufs=1) as persist, \
         tc.tile_pool(name="work", bufs=1) as work, \
         tc.tile_pool(name="ps_a", space="PSUM", bufs=2) as ps_a, \
         tc.tile_pool(name="ps_s", space="PSUM", bufs=2) as ps_s, \
         tc.tile_pool(name="ps_y", space="PSUM", bufs=2) as ps_y:
        # ---------- persistent SBUF ----------
        qt = [persist.tile([128, T], bf16, name=f"qt{m}", tag=f"qt{m}") for m in range(2)]
        kt = [persist.tile([128, TK], bf16, name=f"kt{m}", tag=f"kt{m}") for m in range(2)]
        vsb = persist.tile([128, NTK, HG, 2 * D], bf16, name="vsb", tag="vsb")
        yt = [persist.tile([128, T], bf16, name=f"yt{m}", tag=f"yt{m}") for m in range(2)]
        cmk = persist.tile([128, 2, 128], bf16, name="cmk", tag="cmk")
        cmd = persist.tile([128, 3, 2, 128], bf16, name="cmd", tag="cmd")
        bqs = persist.tile([128, 2], f32, name="bqs", tag="bqs")
        bks = persist.tile([128, 2], f32, name="bks", tag="bks")
        bvb = persist.tile([128, HG, D], f32, name="bvb", tag="bvb")
        wp_t = persist.tile([128, 2, C], bf16, name="wp_t", tag="wp_t")
        wq_t = persist.tile([128, 2, 8, 128], bf16, name="wq_t", tag="wq_t")
        wk_t = persist.tile([128, 2, 8, 128], bf16, name="wk_t", tag="wk_t")
        wv_t = persist.tile([128, 8, CG], bf16, name="wv_t", tag="wv_t")

        # ones block for the AV row sums: generated on-chip
        nc.vector.memset(vsb[:, :, :, D:2 * D], 1.0)

        # pre-load the Act spline tables while the engines wait for data
        # (the first real exp otherwise eats the ~1.3us ACT_TABLE_LOAD)
        actw = work.tile([128, 2], f32, tag="actw", bufs=1)
        nc.vector.memset(actw[:, 0:1], 0.0)
        nc.scalar.activation(actw[:, 1:2], actw[:, 0:1], Exp)

        # SWDGE ring: small constants + V weights + masks (the software DGE
        # has its own descriptor path and surprisingly high burst bandwidth,
        # so it serves as a third lane for window-0-critical bytes)
        nc.gpsimd.dma_start(bqs[:], bq_d[:])
        nc.gpsimd.dma_start(bks[:], bk_d[:])
        nc.gpsimd.dma_start(bvb[:], bvb_d[:])
        nc.gpsimd.dma_start(wv_t[:], wv_d[:])
        nc.gpsimd.dma_start(wk_t[:, 0], wk_d[:, 0])
        nc.gpsimd.dma_start(cmk[:], cmk_d[:])
        nc.gpsimd.dma_start(cmd[:], cmd_d[:])

        # HBM is heavily contended (8 cores start identical DMA streams at
        # once): stream the head-critical bytes across both HWDGE rings in
        # first-use order, with few triggers (each DMA_DIRECT2D costs the
        # issuing engine ~0.6us of NX time).  Per-window x tiles (no ring
        # rotation) keep the trigger engines from blocking on pool reuse.
        xk = []
        xd = []
        for j in range(4):
            xk.append(work.tile([128, 8, KW], bf16, name=f"xk{j}",
                                tag=f"xkw{j}", bufs=1))
            xd.append(work.tile([128, 8, DW], bf16, name=f"xd{j}",
                                tag=f"xdw{j}", bufs=1))
        for k in range(0, 8, 2):
            nc.sync.dma_start(xk[0][:, k:k + 2, :], xk_d[0, :, k:k + 2, :])
        nc.scalar.dma_start(wq_t[:, 0, 0:4], wq_d[:, 0, 0:4])
        nc.scalar.dma_start(xd[0][:, 0:4, :], xd_d[0, :, 0:4, :])
        nc.scalar.dma_start(wq_t[:, 0, 4:8], wq_d[:, 0, 4:8])
        nc.scalar.dma_start(xd[0][:, 4:8, :], xd_d[0, :, 4:8, :])
        # window-0 critical stream ends here; the rest follows in need order
        # mid-kernel loads ride the sync ring only: the scalar-engine NX is
        # the exp pipeline, and every DMA_DIRECT2D costs it ~0.6us
        nc.scalar.dma_start(wk_t[:, 1], wk_d[:, 1])
        nc.sync.dma_start(wq_t[:, 1], wq_d[:, 1])
        nc.sync.dma_start(xk[1][:, 0:4, :], xk_d[1, :, 0:4, :])
        nc.sync.dma_start(xk[1][:, 4:8, :], xk_d[1, :, 4:8, :])
        nc.sync.dma_start(xd[1][:], xd_d[1])
        nc.sync.dma_start(wp_t[:], wp_d[:])
        nc.sync.dma_start(xk[2][:, 0:4, :], xk_d[2, :, 0:4, :])
        nc.sync.dma_start(xk[2][:, 4:8, :], xk_d[2, :, 4:8, :])
        nc.sync.dma_start(xd[2][:], xd_d[2])
        nc.sync.dma_start(xk[3][:, 0:4, :], xk_d[3, :, 0:4, :])
        nc.sync.dma_start(xk[3][:, 4:8, :], xk_d[3, :, 4:8, :])
        nc.sync.dma_start(xd[3][:], xd_d[3])

        # ---------- deferred-emission helpers (PE filler work) ----------
        def emit_q(j, m, pool=None):
            # kept segment [0:KW] and dropped segment [KW:512], one PSUM
            # accumulation group (start on very first, stop on very last)
            pq = (pool or ps_a).tile([128, 512], f32, tag="acc")
            for k in range(8):
                nc.tensor.matmul(pq[:, 0:KW], wq_t[:, m, k, :],
                                 xk[j][:, k, :], start=(k == 0), stop=False)
                nc.tensor.matmul(pq[:, KW:512], wq_t[:, m, k, :],
                                 xd[j][:, k, :], start=False, stop=(k == 7))
            nc.vector.tensor_scalar_add(qt[m][:, 512 * j:512 * (j + 1)],
                                        pq[:], bqs[:, m:m + 1])

        def emit_k(j, m, pool=None):
            # kept tokens only
            pk = (pool or ps_a).tile([128, 512], f32, tag="acc")
            for k in range(8):
                nc.tensor.matmul(pk[:, 0:KW], wk_t[:, m, k, :],
                                 xk[j][:, k, :], start=(k == 0), stop=(k == 7))
            nc.vector.tensor_scalar_add(kt[m][:, KW * j:KW * (j + 1)],
                                        pk[:, 0:KW], bks[:, m:m + 1])

        def emit_v(j, mm, pool=None):
            # kept-token chunk mm as stationary -> vsb chunk 3j+mm directly
            pv = (pool or ps_a).tile([128, 512], f32, tag="acc")
            for k in range(8):
                nc.tensor.matmul(pv[:, 0:CG],
                                 xk[j][:, k, 128 * mm:128 * (mm + 1)],
                                 wv_t[:, k, :], start=(k == 0), stop=(k == 7))
            nc.vector.scalar_tensor_tensor(
                out=vsb[:, 3 * j + mm, :, 0:D],
                in0=pv[:, 0:CG].rearrange("p (h d) -> p h d", d=D),
                scalar=1.0, in1=bvb[:],
                op0=mybir.AluOpType.bypass, op1=mybir.AluOpType.add)

        def emit_qkv_items(j, pools=(None,)):
            fns = [
                lambda p: emit_q(j, 0, p),
                lambda p: emit_k(j, 0, p),
                lambda p: emit_v(j, 0, p),
                lambda p: emit_v(j, 1, p),
                lambda p: emit_q(j, 1, p),
                lambda p: emit_k(j, 1, p),
                lambda p: emit_v(j, 2, p),
            ]
            return [lambda fn=fn, p=pools[ix % len(pools)]: fn(p)
                    for ix, fn in enumerate(fns)]

        def emit_proj(m):
            # output projection for token chunk m; bf16 staged, bf16 store
            stage = work.tile([128, C], bf16, tag="stage", bufs=2)
            for n in range(2):
                po = ps_a.tile([128, 512], f32, tag="acc")
                for k2 in range(2):
                    nc.tensor.matmul(
                        po[:], yt[k2][:, 128 * m:128 * (m + 1)],
                        wp_t[:, k2, 512 * n:512 * (n + 1)],
                        start=(k2 == 0), stop=(k2 == 1))
                nc.vector.tensor_copy(stage[:, 512 * n:512 * (n + 1)], po[:])
            if m % 2:
                ring = nc.gpsimd if m < 12 else nc.scalar
                ring.dma_start(out_d[128 * m:128 * (m + 1), :], stage[:])
            else:
                nc.sync.dma_start(out_d[128 * m:128 * (m + 1), :], stage[:])

        # window-3 projection is split by k2 so only half the matmuls sit
        # behind the final softmax-norm: the k2=0 halves (ready once hp0's
        # norm lands mid-window) run as hp1-pass filler into fp32 stages
        pstage = {}

        def emit_proj_half(m, n):
            po = ps_a.tile([128, 512], f32, tag="acc")
            nc.tensor.matmul(po[:], yt[0][:, 128 * m:128 * (m + 1)],
                             wp_t[:, 0, 512 * n:512 * (n + 1)],
                             start=True, stop=True)
            st = work.tile([128, 512], f32, tag=f"pst{m}_{n}", bufs=1)
            nc.vector.tensor_copy(st[:], po[:])
            pstage[(m, n)] = st

        def emit_proj_fin(m):
            stage = work.tile([128, C], bf16, tag="stage", bufs=2)
            for n in range(2):
                pool = ps_y if n else ps_a
                po = pool.tile([128, 512], f32, tag="pyo" if n else "acc")
                nc.tensor.matmul(po[:], yt[1][:, 128 * m:128 * (m + 1)],
                                 wp_t[:, 1, 512 * n:512 * (n + 1)],
                                 start=True, stop=True)
                nc.vector.scalar_tensor_tensor(
                    out=stage[:, 512 * n:512 * (n + 1)], in0=po[:],
                    scalar=1.0, in1=pstage[(m, n)][:],
                    op0=mybir.AluOpType.bypass, op1=mybir.AluOpType.add)
                ring = nc.sync if (m + n) % 2 == 0 else nc.scalar
                ring.dma_start(out_d[128 * m:128 * (m + 1),
                                     512 * n:512 * (n + 1)],
                               stage[:, 512 * n:512 * (n + 1)])

        # ---------- main schedule ----------
        pending = []
        post_norm = []  # deferred softmax-norm emission from previous window

        def drain_one():
            if pending:
                pending.pop(0)()

        def emit_norm2(hp, scr, jw):
            # deferred norm from an SBUF copy of the AV accumulators: both
            # heads' rowsums in one Ln and one Exp (halves the Act bubble,
            # and keeps the norm out of the producing window's exp stream)
            rec = work.tile([64, 2, 512], f32, tag="rec2", bufs=3)
            lns = work.tile([64, 2, 512], f32, tag="lns2", bufs=3)
            nc.scalar.activation(lns[:], scr[64:128, :, :],
                                 mybir.ActivationFunctionType.Ln)
            nc.scalar.activation(rec[:], lns[:], Exp, bias=0.0, scale=-1.0)
            for q in range(2):
                nc.vector.tensor_tensor(
                    yt[hp][64 * q:64 * q + 64, jw],
                    scr[0:64, q, :], rec[:, q, :], op=MULT)

        def evac_norm(hp, pys, jw, tag):
            # copy the AV accumulators out of PSUM (frees the banks) and
            # return a closure that emits the Act/DVE norm ops later
            scr = work.tile([128, 2, 512], f32, tag=tag, bufs=2)
            for q in range(2):
                nc.vector.tensor_copy(scr[:, q, :], pys[hp][q][:])
            return lambda s=scr, h=hp, j2=jw: emit_norm2(h, s, j2)

        # window 0: emit only what attention(hp0, tile0) needs inline; defer
        # the rest into the attention slots (2 filler drains per tile in
        # window 0)
        w0 = emit_qkv_items(0)
        for it in w0[:3]:
            it()
        pending.extend(w0[3:])

        for j in range(4):
            if j == 0:
                pending.extend(emit_qkv_items(1))
            elif j < 3:
                # qkv filler first (needed by next window's attention), then
                # any deferred output-projection pieces
                pending[0:0] = emit_qkv_items(j + 1)

            jwin = slice(512 * j, 512 * (j + 1))
            ntile = 3 * (j + 1)
            nb0 = ntile - 3  # first boundary tile index
            pys = {}

            def emit_avs(hp, i, pt2, avo):
                for q in range(2):
                    nc.tensor.matmul(
                        pys[hp][q][:, avo:512], vsb[:, i, 2 * hp + q, :],
                        pt2[:, q, avo:512],
                        start=(i == 0), stop=(i == ntile - 1))

            # software-pipelined over a flat (hp, i) tile list: QK(t) and the
            # filler overlap exp(t) on Act; AV trails by one tile
            prev = None
            my_norms = []
            tiles = [(hp, i) for hp in range(2) for i in range(ntile)]
            # emit the previous window's deferred norms a couple of exp
            # tiles into this window so their Act ops don't delay the first
            # exps (window 3 drains yt-readers from tile 0, so emit at 0)
            for tix, (hp, i) in enumerate(tiles):
                if j == 3:
                    if tix == 0:
                        while post_norm:
                            post_norm.pop(0)()
                elif tix in (2, 4) and post_norm:
                    post_norm.pop(0)()
                if i == 0:
                    pys[hp] = [ps_y.tile([128, 512], f32,
                                         name=f"py{j}_{hp}_{q}", tag="pyo")
                               for q in range(2)]
                ps2 = ps_s.tile([128, 2, 512], f32, tag="ps2")
                pt2 = work.tile([128, 2, 512], bf16, tag="pt2", bufs=4)
                u = i - nb0
                # boundary tiles: kept-query prefix [0:128u) is fully masked
                off = 128 * u if u >= 1 else 0
                # first tiles of a window: raise priority so their QK/exp
                # beat the previous window's stragglers and hide the norm
                hoist = (tc.high_priority(offset=150)
                         if (j > 0 and tix < 2) else contextlib.nullcontext())
                with hoist:
                    for q in range(2):  # q: row group (head 2*hp + q)
                        nc.tensor.matmul(
                            ps2[:, q, off:512],
                            kt[hp][64 * q:64 * q + 64, 128 * i:128 * (i + 1)],
                            qt[hp][64 * q:64 * q + 64,
                                   512 * j + off:512 * (j + 1)],
                            start=True, stop=True)
                    nc.scalar.activation(pt2[:, :, off:512],
                                         ps2[:, :, off:512],
                                         Exp, bias=0.0, scale=SCALE)
                if u == 2:  # u=2 AV stays full width: zero skipped cols
                    nc.gpsimd.memset(pt2[:, :, 0:off], 0.0)
                if u >= 0:  # boundary tile: causal mask (both heads at once)
                    # kept partial block [128u:128u+128) is the SAME
                    # triangular mask in compacted coords for every u; the
                    # dropped block [KW:512) is per-u from the host
                    nc.vector.tensor_tensor(
                        pt2[:, :, 128 * u:128 * u + 128],
                        pt2[:, :, 128 * u:128 * u + 128],
                        cmk[:], op=MULT)
                    nc.vector.tensor_tensor(
                        pt2[:, :, KW:512], pt2[:, :, KW:512],
                        cmd[:, u], op=MULT)
                drain_one()
                if j == 0:
                    drain_one()
                if prev is not None:
                    emit_avs(*prev)
                    if prev[1] == ntile - 1:
                        my_norms.append(evac_norm(0, pys, jwin, "scr0"))
                prev = (hp, i, pt2, 128 if u == 1 else 0)
            emit_avs(*prev)
            if j < 3:
                my_norms.append(evac_norm(1, pys, jwin, "scr1"))
                post_norm.extend(my_norms)
            else:
                # final window: hp0's norm Act ops go out BEFORE the hp1
                # evacuation so coarse engine-counter sems don't chain them
                # behind it; the k2=0 projection halves fill the PE while
                # hp1's norm resolves
                my_norms[0]()
                evac_norm(1, pys, jwin, "scr1")()
                for m in range(12, 16):
                    for n in range(2):
                        emit_proj_half(m, n)
            while pending:
                drain_one()
            # ---- output projection for the finished query window ----
            if j < 3:
                pending.extend(
                    [lambda m=m: emit_proj(m) for m in range(4 * j, 4 * j + 4)])
            else:
                for m in range(12, 16):
                    emit_proj_fin(m)

    if split:
        _split_multi_waits(nc, mybir)
    return nc


def _get_nc():
    if "nc" not in _CACHE:
        _CACHE["nc"] = _build_bass()
    return _CACHE["nc"]


def _tokperm():
    # per-window permutation: [384 kept | 128 dropped] -> original index
    p = np.arange(512)
    kept = (p // 3) * 4 + p % 3          # for p < 384
    drop = (p - 384) * 4 + 3             # for p >= 384
    perm = np.where(p < KW, kept, drop)
    full = (np.arange(4)[:, None] * 512 + perm[None, :]).reshape(-1)
    return full  # [T] device row r holds token full[r]


def _host_maps(inputs):
    x = np.asarray(inputs["x"], np.float32)
    Wq = np.asarray(inputs["Wq"], np.float32)
    Wk = np.asarray(inputs["Wk"], np.float32)
    Wv = np.asarray(inputs["Wv"], np.float32)
    Wp = np.asarray(inputs["Wp"], np.float32)
    bq = np.asarray(inputs["bq"], np.float32)
    bk = np.asarray(inputs["bk"], np.float32)
    bv = np.asarray(inputs["bv"], np.float32)

    # causal masks in permuted-window coordinates: the kept x kept partial
    # block is the same lower-triangular mask (in compacted coords) for
    # every boundary chunk u; the dropped-query block differs per u.
    kp = np.arange(128)
    dq = np.arange(DW) * 4 + 3  # dropped-query original (window-local) index
    tri = (kp[None, :] >= kp[:, None]).astype(np.float32)
    cmk = np.zeros((128, 2, 128), np.float32)
    cmk[:, 0] = tri
    cmk[:, 1] = tri
    cmd = np.zeros((128, 3, 2, 128), np.float32)
    for u in range(3):
        ko = ((128 * u + kp) // 3) * 4 + (128 * u + kp) % 3  # key orig index
        dm = (dq[None, :] >= ko[:, None]).astype(np.float32)
        cmd[:, u, 0] = dm
        cmd[:, u, 1] = dm

    # permuted, compacted x: xk = kept tokens, xd = dropped tokens, both
    # partition-major [window, 128 cin-in-chunk, 8 cin-chunk, tokens]
    keep = np.arange(T) % 4 != 3
    xks, xds = [], []
    for b in range(B):
        xw = x[b].reshape(4, 512, C)
        xkb = np.stack([xw[j][keep[:512]] for j in range(4)])   # [4,384,C]
        xdb = np.stack([xw[j][~keep[:512]] for j in range(4)])  # [4,128,C]
        xks.append(np.ascontiguousarray(
            xkb.transpose(0, 2, 1).reshape(4, 8, 128, KW).transpose(0, 2, 1, 3)
        ).astype(ml_dtypes.bfloat16))
        xds.append(np.ascontiguousarray(
            xdb.transpose(0, 2, 1).reshape(4, 8, 128, DW).transpose(0, 2, 1, 3)
        ).astype(ml_dtypes.bfloat16))
    maps = []
    for c in range(8):
        b, g = c // 4, c % 4
        sl = slice(CG * g, CG * (g + 1))
        maps.append({
            "xk": xks[b],
            "xd": xds[b],
            "wq": np.ascontiguousarray(
                Wq[:, sl].reshape(8, 128, 2, 128).transpose(1, 2, 0, 3)
            ).astype(ml_dtypes.bfloat16),
            "wk": np.ascontiguousarray(
                Wk[:, sl].reshape(8, 128, 2, 128).transpose(1, 2, 0, 3)
            ).astype(ml_dtypes.bfloat16),
            "wv": np.ascontiguousarray(
                Wv[:, sl].reshape(8, 128, CG).transpose(1, 0, 2)
            ).astype(ml_dtypes.bfloat16),
            "wp": np.ascontiguousarray(
                Wp[sl, :].reshape(2, 128, C).transpose(1, 0, 2)
            ).astype(ml_dtypes.bfloat16),
            "bq2": np.ascontiguousarray(bq[sl].reshape(2, 128).T),
            "bk2": np.ascontiguousarray(bk[sl].reshape(2, 128).T),
            "bvb": np.ascontiguousarray(
                np.broadcast_to(bv[sl].reshape(HG, D), (128, HG, D))),
            "cmk": cmk.astype(ml_dtypes.bfloat16),
            "cmd": cmd.astype(ml_dtypes.bfloat16),
        })
    return maps


def _combine(results, inputs):
    bp = np.asarray(inputs["bp"], np.float32)
    perm = _tokperm()
    out = np.zeros((B, T, C), np.float32)
    for c in range(8):
        out[c // 4, perm] += np.asarray(results[c]["out"]).astype(np.float32)
    out += bp[None, None, :]
    return out


def _run(inputs, profile_dir=None, trace_cores=None):
    nc = _get_nc()
    maps = _host_maps(inputs)
    from concourse.bass_utils import run_bass_kernel_spmd
    if profile_dir is not None:
        import types, sys
        from trn_agent_boot.trn_boot import _ntff_profile_via_ctypes
        hook = _ntff_profile_via_ctypes("/opt/axon/libaxon_pjrt.so")
        with hook(profile_dir, trace_cores or [0]):
            res = run_bass_kernel_spmd(nc, maps, core_ids=list(range(8)))
    else:
        res = run_bass_kernel_spmd(nc, maps, core_ids=list(range(8)))
    return _combine(res.results, inputs)


def kernel(**inputs):
    return _run(inputs)


# revision 35
# speedup vs baseline: 1.0375x; 1.0320x over previous
# Trainium2 Bass kernel for CausalSelfAttention (B=2, T=2048, C=1024, H=16, D=64)
# with periodic mask: causal AND (key_col % 4 != 3).
#
# Sharding (8 NeuronCores): core c = (b, g) with b = c//4 (batch), g = c%4
# (head group of 4 heads). Each core computes QKV for its 4 heads, attention,
# and a partial output projection y_heads @ Wp[rows]. Host sums the 4 partials
# per batch and adds bp (tensor-parallel reduce).
#
# Key device-side choices:
#  - Token permutation: each 512-token window is reordered on the host as
#    [384 kept | 128 dropped] (kept = t%4 != 3).  K and V projections run
#    only on the kept segment (25% less PE work, no gather matmuls), and the
#    kept x kept causal mask is EXACTLY lower-triangular in compacted
#    coordinates.  Q runs on both segments (two moving ranges into one PSUM
#    accumulation, same cycle count as dense).  The host un-permutes the
#    output rows at combine time.
#  - All DRAM inputs are pre-arranged on the host into the exact SBUF layout
#    (partition-major), so every bulk DMA is a contiguous per-partition
#    stream. The first weight/x pieces are split into k-chunk grains across
#    both HWDGE rings so the first QKV matmul can start as early as possible.
#  - Scores are produced transposed (S^T[tk_kept, tq]) so softmax-normalized
#    probabilities feed the AV matmul directly as the moving operand.
#  - Softmax row sums come from a 64-wide all-ones block in the V tiles; the
#    reciprocal runs on DVE (nc.vector.reciprocal), keeping the Act engine
#    free for the score exponentials (its true workload).
#  - Emission interleave: QKV chains of window j+1 and output-projection
#    pieces of window j-1 are woven between attention tiles of window j so
#    the PE keeps busy while the Act engine works through the exps.
#  - Output projection stores DMA straight from a bf16 stage.

import ml_dtypes
import numpy as np

B, T, C, H, D = 2, 2048, 1024, 16, 64
HG = 4          # heads per core
CG = HG * D     # = 256 columns of C per core
TK = (T // 4) * 3   # 1536 kept key positions
NTK = TK // 128     # 12 kept-key chunks of 128
KW = 384            # kept tokens per 512-token window
DW = 128            # dropped tokens per window
SCALE = 1.0 / 8.0   # 1/sqrt(D)

_CACHE = {}


def _split_multi_waits(nc, mybir):
    # The pinned walrus here encodes at most 1 sync-wait per instruction
    # (2 for EventSemaphore). Hoist excess waits onto standalone NoOps that
    # precede the instruction on the same engine.
    f = nc.m.functions[0]
    n = 0
    for b in f.blocks:
        insts = list(b.instructions)
        out = []
        changed = False
        for inst in insts:
            si = inst.sync_info
            if si is not None:
                waits = list(si.on_wait)
                cap = 2 if isinstance(inst, mybir.InstEventSemaphore) else 1
                if len(waits) > cap:
                    for w in waits[cap:]:
                        out.append(mybir.InstNoOp(
                            name=f"{inst.name}-ws{n}", engine=inst.engine,
                            ins=[], outs=[],
                            sync_info=mybir.SyncInfo(on_wait=[w], on_update=[])))
                        n += 1
                    inst.sync_info = mybir.SyncInfo(
                        on_wait=waits[:cap], on_update=list(si.on_update))
                    changed = True
            out.append(inst)
        if changed:
            b.instructions = out
    return n


def _build_bass(split=True):
    import contextlib
    import concourse.bass as bass
    import concourse.tile as tile
    import concourse.mybir as mybir

    f32 = mybir.dt.float32
    bf16 = mybir.dt.bfloat16

    nc = bass.Bass("TRN2", debug=False, num_devices=8)

    # host-prearranged, partition-major
    xk_d = nc.dram_tensor("xk", [4, 128, 8, KW], bf16, kind="ExternalInput").ap()
    xd_d = nc.dram_tensor("xd", [4, 128, 8, DW], bf16, kind="ExternalInput").ap()
    wq_d = nc.dram_tensor("wq", [128, 2, 8, 128], bf16, kind="ExternalInput").ap()
    wk_d = nc.dram_tensor("wk", [128, 2, 8, 128], bf16, kind="ExternalInput").ap()
    wv_d = nc.dram_tensor("wv", [128, 8, CG], bf16, kind="ExternalInput").ap()
    wp_d = nc.dram_tensor("wp", [128, 2, C], bf16, kind="ExternalInput").ap()
    bq_d = nc.dram_tensor("bq2", [128, 2], f32, kind="ExternalInput").ap()
    bk_d = nc.dram_tensor("bk2", [128, 2], f32, kind="ExternalInput").ap()
    bvb_d = nc.dram_tensor("bvb", [128, HG, D], f32, kind="ExternalInput").ap()
    cmk_d = nc.dram_tensor("cmk", [128, 2, 128], bf16, kind="ExternalInput").ap()
    cmd_d = nc.dram_tensor("cmd", [128, 3, 2, 128], bf16, kind="ExternalInput").ap()
    out_d = nc.dram_tensor("out", [T, C], bf16, kind="ExternalOutput").ap()

    Exp = mybir.ActivationFunctionType.Exp
    MULT = mybir.AluOpType.mult

    with tile.TileContext(nc) as tc, \
         tc.tile_pool(name="persist", bufs=1) as persist, \
         tc.tile_pool(name="work", bufs=1) as work, \
         tc.tile_pool(name="ps_a", space="PSUM", bufs=2) as ps_a, \
         tc.tile_pool(name="ps_s", space="PSUM", bufs=2) as ps_s, \
         tc.tile_pool(name="ps_y", space="PSUM", bufs=2) as ps_y:
        # ---------- persistent SBUF ----------
        qt = [persist.tile([128, T], bf16, name=f"qt{m}", tag=f"qt{m}") for m in range(2)]
        kt = [persist.tile([128, TK], bf16, name=f"kt{m}", tag=f"kt{m}") for m in range(2)]
        vsb = persist.tile([128, NTK, HG, 2 * D], bf16, name="vsb", tag="vsb")
        yt = [persist.tile([128, T], bf16, name=f"yt{m}", tag=f"yt{m}") for m in range(2)]
        cmk = persist.tile([128, 2, 128], bf16, name="cmk", tag="cmk")
        cmd = persist.tile([128, 3, 2, 128], bf16, name="cmd", tag="cmd")
        bqs = persist.tile([128, 2], f32, name="bqs", tag="bqs")
        bks = persist.tile([128, 2], f32, name="bks", tag="bks")
        bvb = persist.tile([128, HG, D], f32, name="bvb", tag="bvb")
        wp_t = persist.tile([128, 2, C], bf16, name="wp_t", tag="wp_t")
        wq_t = persist.tile([128, 2, 8, 128], bf16, name="wq_t", tag="wq_t")
        wk_t = persist.tile([128, 2, 8, 128], bf16, name="wk_t", tag="wk_t")
        wv_t = persist.tile([128, 8, CG], bf16, name="wv_t", tag="wv_t")

        # ones block for the AV row sums: generated on-chip
        nc.vector.memset(vsb[:, :, :, D:2 * D], 1.0)

        # pre-load the Act spline tables while the engines wait for data
        # (the first real exp otherwise eats the ~1.3us ACT_TABLE_LOAD)
        actw = work.tile([128, 2], f32, tag="actw", bufs=1)
        nc.vector.memset(actw[:, 0:1], 0.0)
        nc.scalar.activation(actw[:, 1:2], actw[:, 0:1], Exp)

        # SWDGE ring: small constants + V weights + masks (the software DGE
        # has its own descriptor path and surprisingly high burst bandwidth,
        # so it serves as a third lane for window-0-critical bytes)
        nc.gpsimd.dma_start(bqs[:], bq_d[:])
        nc.gpsimd.dma_start(bks[:], bk_d[:])
        nc.gpsimd.dma_start(bvb[:], bvb_d[:])
        nc.gpsimd.dma_start(wv_t[:], wv_d[:])
        nc.gpsimd.dma_start(wk_t[:, 0], wk_d[:, 0])
        nc.gpsimd.dma_start(cmk[:], cmk_d[:])
        nc.gpsimd.dma_start(cmd[:], cmd_d[:])

        # HBM is heavily contended (8 cores start identical DMA streams at
        # once): stream the head-critical bytes across both HWDGE rings in
        # first-use order, with few triggers (each DMA_DIRECT2D costs the
        # issuing engine ~0.6us of NX time).  Per-window x tiles (no ring
        # rotation) keep the trigger engines from blocking on pool reuse.
        xk = []
        xd = []
        for j in range(4):
            xk.append(work.tile([128, 8, KW], bf16, name=f"xk{j}",
                                tag=f"xkw{j}", bufs=1))
            xd.append(work.tile([128, 8, DW], bf16, name=f"xd{j}",
                                tag=f"xdw{j}", bufs=1))
        for k in range(0, 8, 2):
            nc.sync.dma_start(xk[0][:, k:k + 2, :], xk_d[0, :, k:k + 2, :])
        nc.scalar.dma_start(wq_t[:, 0, 0:4], wq_d[:, 0, 0:4])
        nc.scalar.dma_start(xd[0][:, 0:4, :], xd_d[0, :, 0:4, :])
        nc.scalar.dma_start(wq_t[:, 0, 4:8], wq_d[:, 0, 4:8])
        nc.scalar.dma_start(xd[0][:, 4:8, :], xd_d[0, :, 4:8, :])
        # window-0 critical stream ends here; the rest follows in need order
        # mid-kernel loads ride the sync ring only: the scalar-engine NX is
        # the exp pipeline, and every DMA_DIRECT2D costs it ~0.6us
        nc.scalar.dma_start(wk_t[:, 1], wk_d[:, 1])
        nc.sync.dma_start(wq_t[:, 1], wq_d[:, 1])
        nc.sync.dma_start(xk[1][:, 0:4, :], xk_d[1, :, 0:4, :])
        nc.sync.dma_start(xk[1][:, 4:8, :], xk_d[1, :, 4:8, :])
        nc.sync.dma_start(xd[1][:], xd_d[1])
        nc.sync.dma_start(wp_t[:], wp_d[:])
        nc.sync.dma_start(xk[2][:, 0:4, :], xk_d[2, :, 0:4, :])
        nc.sync.dma_start(xk[2][:, 4:8, :], xk_d[2, :, 4:8, :])
        nc.sync.dma_start(xd[2][:], xd_d[2])
        nc.sync.dma_start(xk[3][:, 0:4, :], xk_d[3, :, 0:4, :])
        nc.sync.dma_start(xk[3][:, 4:8, :], xk_d[3, :, 4:8, :])
        nc.sync.dma_start(xd[3][:], xd_d[3])

        # ---------- deferred-emission helpers (PE filler work) ----------
        def emit_q(j, m, pool=None):
            # kept segment [0:KW] and dropped segment [KW:512], one PSUM
            # accumulation group (start on very first, stop on very last)
            pq = (pool or ps_a).tile([128, 512], f32, tag="acc")
            for k in range(8):
                nc.tensor.matmul(pq[:, 0:KW], wq_t[:, m, k, :],
                                 xk[j][:, k, :], start=(k == 0), stop=False)
                nc.tensor.matmul(pq[:, KW:512], wq_t[:, m, k, :],
                                 xd[j][:, k, :], start=False, stop=(k == 7))
            nc.vector.tensor_scalar_add(qt[m][:, 512 * j:512 * (j + 1)],
                                        pq[:], bqs[:, m:m + 1])

        def emit_k(j, m, pool=None):
            # kept tokens only
            pk = (pool or ps_a).tile([128, 512], f32, tag="acc")
            for k in range(8):
                nc.tensor.matmul(pk[:, 0:KW], wk_t[:, m, k, :],
                                 xk[j][:, k, :], start=(k == 0), stop=(k == 7))
            nc.vector.tensor_scalar_add(kt[m][:, KW * j:KW * (j + 1)],
                                        pk[:, 0:KW], bks[:, m:m + 1])

        def emit_v(j, mm, pool=None):
            # kept-token chunk mm as stationary -> vsb chunk 3j+mm directly
            pv = (pool or ps_a).tile([128, 512], f32, tag="acc")
            for k in range(8):
                nc.tensor.matmul(pv[:, 0:CG],
                                 xk[j][:, k, 128 * mm:128 * (mm + 1)],
                                 wv_t[:, k, :], start=(k == 0), stop=(k == 7))
            nc.vector.scalar_tensor_tensor(
                out=vsb[:, 3 * j + mm, :, 0:D],
                in0=pv[:, 0:CG].rearrange("p (h d) -> p h d", d=D),
                scalar=1.0, in1=bvb[:],
                op0=mybir.AluOpType.bypass, op1=mybir.AluOpType.add)

        def emit_qkv_items(j, pools=(None,)):
            fns = [
                lambda p: emit_q(j, 0, p),
                lambda p: emit_k(j, 0, p),
                lambda p: emit_v(j, 0, p),
                lambda p: emit_v(j, 1, p),
                lambda p: emit_q(j, 1, p),
                lambda p: emit_k(j, 1, p),
                lambda p: emit_v(j, 2, p),
            ]
            return [lambda fn=fn, p=pools[ix % len(pools)]: fn(p)
                    for ix, fn in enumerate(fns)]

        def emit_proj(m):
            # output projection for token chunk m; bf16 staged, bf16 store
            stage = work.tile([128, C], bf16, tag="stage", bufs=2)
            for n in range(2):
                po = ps_a.tile([128, 512], f32, tag="acc")
                for k2 in range(2):
                    nc.tensor.matmul(
                        po[:], yt[k2][:, 128 * m:128 * (m + 1)],
                        wp_t[:, k2, 512 * n:512 * (n + 1)],
                        start=(k2 == 0), stop=(k2 == 1))
                nc.vector.tensor_copy(stage[:, 512 * n:512 * (n + 1)], po[:])
            if m % 2:
                ring = nc.gpsimd if m < 12 else nc.scalar
                ring.dma_start(out_d[128 * m:128 * (m + 1), :], stage[:])
            else:
                nc.sync.dma_start(out_d[128 * m:128 * (m + 1), :], stage[:])

        # window-3 projection is split by k2 so only half the matmuls sit
        # behind the final softmax-norm: the k2=0 halves (ready once hp0's
        # norm lands mid-window) run as hp1-pass filler into fp32 stages
        pstage = {}

        def emit_proj_half(m, n):
            po = ps_a.tile([128, 512], f32, tag="acc")
            nc.tensor.matmul(po[:], yt[0][:, 128 * m:128 * (m + 1)],
                             wp_t[:, 0, 512 * n:512 * (n + 1)],
                             start=True, stop=True)
            st = work.tile([128, 512], f32, tag=f"pst{m}_{n}", bufs=1)
            nc.vector.tensor_copy(st[:], po[:])
            pstage[(m, n)] = st

        def emit_proj_fin(m):
            stage = work.tile([128, C], bf16, tag="stage", bufs=2)
            for n in range(2):
                pool = ps_y if n else ps_a
                po = pool.tile([128, 512], f32, tag="pyo" if n else "acc")
                nc.tensor.matmul(po[:], yt[1][:, 128 * m:128 * (m + 1)],
                                 wp_t[:, 1, 512 * n:512 * (n + 1)],
                                 start=True, stop=True)
                nc.vector.scalar_tensor_tensor(
                    out=stage[:, 512 * n:512 * (n + 1)], in0=po[:],
                    scalar=1.0, in1=pstage[(m, n)][:],
                    op0=mybir.AluOpType.bypass, op1=mybir.AluOpType.add)
                ring = nc.sync if (m + n) % 2 == 0 else nc.scalar
                ring.dma_start(out_d[128 * m:128 * (m + 1),
                                     512 * n:512 * (n + 1)],
                               stage[:, 512 * n:512 * (n + 1)])

        # ---------- main schedule ----------
        pending = []
        post_norm = []  # deferred softmax-norm emission from previous window

        def drain_one():
            if pending:
                pending.pop(0)()

        def emit_norm2(hp, scr, jw):
            # deferred norm from an SBUF copy of the AV accumulators: both
            # heads' rowsums in one Ln and one Exp (halves the Act bubble,
            # and keeps the norm out of the producing window's exp stream)
            rec = work.tile([64, 2, 512], f32, tag="rec2", bufs=3)
            lns = work.tile([64, 2, 512], f32, tag="lns2", bufs=3)
            nc.scalar.activation(lns[:], scr[64:128, :, :],
                                 mybir.ActivationFunctionType.Ln)
            nc.scalar.activation(rec[:], lns[:], Exp, bias=0.0, scale=-1.0)
            for q in range(2):
                nc.vector.tensor_tensor(
                    yt[hp][64 * q:64 * q + 64, jw],
                    scr[0:64, q, :], rec[:, q, :], op=MULT)

        def evac_norm(hp, pys, jw, tag):
            # copy the AV accumulators out of PSUM (frees the banks) and
            # return a closure that emits the Act/DVE norm ops later
            scr = work.tile([128, 2, 512], f32, tag=tag, bufs=2)
            for q in range(2):
                nc.vector.tensor_copy(scr[:, q, :], pys[hp][q][:])
            return lambda s=scr, h=hp, j2=jw: emit_norm2(h, s, j2)

        # window 0: emit only what attention(hp0, tile0) needs inline; defer
        # the rest into the attention slots (2 filler drains per tile in
        # window 0)
        w0 = emit_qkv_items(0)
        for it in w0[:3]:
            it()
        pending.extend(w0[3:])

        for j in range(4):
            if j == 0:
                pending.extend(emit_qkv_items(1))
            elif j < 3:
                # qkv filler first (needed by next window's attention), then
                # any deferred output-projection pieces
                pending[0:0] = emit_qkv_items(j + 1)

            jwin = slice(512 * j, 512 * (j + 1))
            ntile = 3 * (j + 1)
            nb0 = ntile - 3  # first boundary tile index
            pys = {}

            def emit_avs(hp, i, pt2, avo):
                for q in range(2):
                    nc.tensor.matmul(
                        pys[hp][q][:, avo:512], vsb[:, i, 2 * hp + q, :],
                        pt2[:, q, avo:512],
                        start=(i == 0), stop=(i == ntile - 1))

            # software-pipelined over a flat (hp, i) tile list: QK(t) and the
            # filler overlap exp(t) on Act; AV trails by one tile
            prev = None
            my_norms = []
            tiles = [(hp, i) for hp in range(2) for i in range(ntile)]
            nt2 = len(tiles)
            drain0 = len(pending)
            drained = 0
            # emit the previous window's deferred norms a couple of exp
            # tiles into this window so their Act ops don't delay the first
            # exps (window 3 drains yt-readers from tile 0, so emit at 0)
            for tix, (hp, i) in enumerate(tiles):
                if j == 3:
                    if tix == 0:
                        while post_norm:
                            post_norm.pop(0)()
                elif tix in (2, 4) and post_norm:
                    post_norm.pop(0)()
                if i == 0:
                    pys[hp] = [ps_y.tile([128, 512], f32,
                                         name=f"py{j}_{hp}_{q}", tag="pyo")
                               for q in range(2)]
                ps2 = ps_s.tile([128, 2, 512], f32, tag="ps2")
                pt2 = work.tile([128, 2, 512], bf16, tag="pt2", bufs=4)
                u = i - nb0
                # boundary tiles: kept-query prefix [0:128u) is fully masked
                off = 128 * u if u >= 1 else 0
                # first tiles of a window: raise priority so their QK/exp
                # beat the previous window's stragglers and hide the norm
                hoist = (tc.high_priority(offset=150)
                         if (j > 0 and tix < 2) else contextlib.nullcontext())
                with hoist:
                    for q in range(2):  # q: row group (head 2*hp + q)
                        nc.tensor.matmul(
                            ps2[:, q, off:512],
                            kt[hp][64 * q:64 * q + 64, 128 * i:128 * (i + 1)],
                            qt[hp][64 * q:64 * q + 64,
                                   512 * j + off:512 * (j + 1)],
                            start=True, stop=True)
                    nc.scalar.activation(pt2[:, :, off:512],
                                         ps2[:, :, off:512],
                                         Exp, bias=0.0, scale=SCALE)
                if u == 2:  # u=2 AV stays full width: zero skipped cols
                    nc.gpsimd.memset(pt2[:, :, 0:off], 0.0)
                if u >= 0:  # boundary tile: causal mask (both heads at once)
                    # kept partial block [128u:128u+128) is the SAME
                    # triangular mask in compacted coords for every u; the
                    # dropped block [KW:512) is per-u from the host
                    nc.vector.tensor_tensor(
                        pt2[:, :, 128 * u:128 * u + 128],
                        pt2[:, :, 128 * u:128 * u + 128],
                        cmk[:], op=MULT)
                    nc.vector.tensor_tensor(
                        pt2[:, :, KW:512], pt2[:, :, KW:512],
                        cmd[:, u], op=MULT)
                # spread filler emission evenly across the window: the PE
                # executes ~in emission order, and the late (Act-paced)
                # tiles are where it needs independent work most
                if j == 0:
                    drain_one()
                    drain_one()
                else:
                    target = ((tix + 1) * drain0 + nt2 - 1) // nt2
                    while drained < target and pending:
                        drain_one()
                        drained += 1
                if prev is not None:
                    emit_avs(*prev)
                    if prev[1] == ntile - 1:
                        my_norms.append(evac_norm(0, pys, jwin, "scr0"))
                prev = (hp, i, pt2, 128 if u == 1 else 0)
            emit_avs(*prev)
            if j < 3:
                my_norms.append(evac_norm(1, pys, jwin, "scr1"))
                post_norm.extend(my_norms)
            else:
                # final window: hp0's norm Act ops go out BEFORE the hp1
                # evacuation so coarse engine-counter sems don't chain them
                # behind it; the k2=0 projection halves fill the PE while
                # hp1's norm resolves
                my_norms[0]()
                evac_norm(1, pys, jwin, "scr1")()
                for m in range(12, 16):
                    for n in range(2):
                        emit_proj_half(m, n)
            while pending:
                drain_one()
            # ---- output projection for the finished query window ----
            if j < 3:
                pending.extend(
                    [lambda m=m: emit_proj(m) for m in range(4 * j, 4 * j + 4)])
            else:
                for m in range(12, 16):
                    emit_proj_fin(m)

    if split:
        _split_multi_waits(nc, mybir)
    return nc


def _get_nc():
    if "nc" not in _CACHE:
        _CACHE["nc"] = _build_bass()
    return _CACHE["nc"]


def _tokperm():
    # per-window permutation: [384 kept | 128 dropped] -> original index
    p = np.arange(512)
    kept = (p // 3) * 4 + p % 3          # for p < 384
    drop = (p - 384) * 4 + 3             # for p >= 384
    perm = np.where(p < KW, kept, drop)
    full = (np.arange(4)[:, None] * 512 + perm[None, :]).reshape(-1)
    return full  # [T] device row r holds token full[r]


def _host_maps(inputs):
    x = np.asarray(inputs["x"], np.float32)
    Wq = np.asarray(inputs["Wq"], np.float32)
    Wk = np.asarray(inputs["Wk"], np.float32)
    Wv = np.asarray(inputs["Wv"], np.float32)
    Wp = np.asarray(inputs["Wp"], np.float32)
    bq = np.asarray(inputs["bq"], np.float32)
    bk = np.asarray(inputs["bk"], np.float32)
    bv = np.asarray(inputs["bv"], np.float32)

    # causal masks in permuted-window coordinates: the kept x kept partial
    # block is the same lower-triangular mask (in compacted coords) for
    # every boundary chunk u; the dropped-query block differs per u.
    kp = np.arange(128)
    dq = np.arange(DW) * 4 + 3  # dropped-query original (window-local) index
    tri = (kp[None, :] >= kp[:, None]).astype(np.float32)
    cmk = np.zeros((128, 2, 128), np.float32)
    cmk[:, 0] = tri
    cmk[:, 1] = tri
    cmd = np.zeros((128, 3, 2, 128), np.float32)
    for u in range(3):
        ko = ((128 * u + kp) // 3) * 4 + (128 * u + kp) % 3  # key orig index
        dm = (dq[None, :] >= ko[:, None]).astype(np.float32)
        cmd[:, u, 0] = dm
        cmd[:, u, 1] = dm

    # permuted, compacted x: xk = kept tokens, xd = dropped tokens, both
    # partition-major [window, 128 cin-in-chunk, 8 cin-chunk, tokens]
    keep = np.arange(T) % 4 != 3
    xks, xds = [], []
    for b in range(B):
        xw = x[b].reshape(4, 512, C)
        xkb = np.stack([xw[j][keep[:512]] for j in range(4)])   # [4,384,C]
        xdb = np.stack([xw[j][~keep[:512]] for j in range(4)])  # [4,128,C]
        xks.append(np.ascontiguousarray(
            xkb.transpose(0, 2, 1).reshape(4, 8, 128, KW).transpose(0, 2, 1, 3)
        ).astype(ml_dtypes.bfloat16))
        xds.append(np.ascontiguousarray(
            xdb.transpose(0, 2, 1).reshape(4, 8, 128, DW).transpose(0, 2, 1, 3)
        ).astype(ml_dtypes.bfloat16))
    maps = []
    for c in range(8):
        b, g = c // 4, c % 4
        sl = slice(CG * g, CG * (g + 1))
        maps.append({
            "xk": xks[b],
            "xd": xds[b],
            "wq": np.ascontiguousarray(
                Wq[:, sl].reshape(8, 128, 2, 128).transpose(1, 2, 0, 3)
            ).astype(ml_dtypes.bfloat16),
            "wk": np.ascontiguousarray(
                Wk[:, sl].reshape(8, 128, 2, 128).transpose(1, 2, 0, 3)
            ).astype(ml_dtypes.bfloat16),
            "wv": np.ascontiguousarray(
                Wv[:, sl].reshape(8, 128, CG).transpose(1, 0, 2)
            ).astype(ml_dtypes.bfloat16),
            "wp": np.ascontiguousarray(
                Wp[sl, :].reshape(2, 128, C).transpose(1, 0, 2)
            ).astype(ml_dtypes.bfloat16),
            "bq2": np.ascontiguousarray(bq[sl].reshape(2, 128).T),
            "bk2": np.ascontiguousarray(bk[sl].reshape(2, 128).T),
            "bvb": np.ascontiguousarray(
                np.broadcast_to(bv[sl].reshape(HG, D), (128, HG, D))),
            "cmk": cmk.astype(ml_dtypes.bfloat16),
            "cmd": cmd.astype(ml_dtypes.bfloat16),
        })
    return maps


def _combine(results, inputs):
    bp = np.asarray(inputs["bp"], np.float32)
    perm = _tokperm()
    out = np.zeros((B, T, C), np.float32)
    for c in range(8):
        out[c // 4, perm] += np.asarray(results[c]["out"]).astype(np.float32)
    out += bp[None, None, :]
    return out


def _run(inputs, profile_dir=None, trace_cores=None):
    nc = _get_nc()
    maps = _host_maps(inputs)
    from concourse.bass_utils import run_bass_kernel_spmd
    if profile_dir is not None:
        import types, sys
        from trn_agent_boot.trn_boot import _ntff_profile_via_ctypes
        hook = _ntff_profile_via_ctypes("/opt/axon/libaxon_pjrt.so")
        with hook(profile_dir, trace_cores or [0]):
            res = run_bass_kernel_spmd(nc, maps, core_ids=list(range(8)))
    else:
        res = run_bass_kernel_spmd(nc, maps, core_ids=list(range(8)))
    return _combine(res.results, inputs)


def kernel(**inputs):
    return _run(inputs)
